# revision 1
# baseline (speedup 1.0000x reference)
"""Trainium2 Bass kernel for the Consis_Reg MSE loss.

Reference semantics (N=8192, D=512, C=64 classes):
    S[i,j]    = ||a_i - a_j||^2
    per_row_i = sum_{j: t_j == t_i} S[i,j] / cnt_{t_i}
    loss      = sum_i per_row_i

Class-aggregation identity (exact in real arithmetic):
    sum_{j in c} S[i,j] = cnt_c * ||a_i||^2 + sumSq_c - 2 a_i . sumA_c
    =>  loss = 2 * ( total_sumsq - sum_c ||sumA_c||^2 / cnt_c )
where, per class c:
    sumA_c  = sum_{i in c} a_i          (vector in R^D)
    cnt_c   = |{i : t_i == c}|
and total_sumsq = sum_i ||a_i||^2.

Each of the 8 cores processes a 1024-row shard of A:
    osum [64, 512] = M^T @ A_shard   (M = one-hot of targets; PSUM-accumulated
                                      float32r matmuls run at full PE speed and
                                      their tf32-like rounding only perturbs
                                      the small ||sumA_c||^2 correction term)
    ocnt [1, 64]   = per-class count (DVE reduce + GpSimd partition reduce)
    osq  [1, 1]    = sum of squares of the shard, computed in exact fp32 by
                     bitcasting the f32r bytes back to f32 on the DVE/GpSimd
The host sums the 8 partials and folds them into the final scalar.

Rows are assigned to SBUF partitions in contiguous blocks (partition p gets
rows p*8..p*8+7 of the shard) so input DMAs move 8KB-contiguous chunks per
partition; the matmul contraction is invariant to row order because the
one-hot rows are permuted identically.
"""

import numpy as np

N, D, C = 8192, 512, 64
NCORES = 8
ROWS = N // NCORES  # rows per core
P = 128             # SBUF partitions
NT = ROWS // P      # row-tiles per core (rows per partition)
NQ = 4              # input DMA / sumsq chunks
QT = NT // NQ       # row-tiles per chunk

_PROGRAM_CACHE = {}


def _build_program():
    import concourse.bass as bass
    import concourse.bacc as bacc
    import concourse.tile as tile
    from concourse import mybir

    f32 = mybir.dt.float32
    f32r = mybir.dt.float32r
    i32 = mybir.dt.int32

    nc = bacc.Bacc(
        "TRN2", target_bir_lowering=False, debug=False, num_devices=NCORES
    )
    a_dram = nc.dram_tensor("a", [P, NT, D], f32r, kind="ExternalInput").ap()
    t_dram = nc.dram_tensor("t", [P, NT], i32, kind="ExternalInput").ap()
    osum = nc.dram_tensor("osum", [C, D], f32, kind="ExternalOutput").ap()
    ocnt = nc.dram_tensor("ocnt", [P, C], f32, kind="ExternalOutput").ap()
    osq = nc.dram_tensor("osq", [P, NQ], f32, kind="ExternalOutput").ap()

    with tile.TileContext(nc) as tc:
        with (
            tc.tile_pool(name="big", bufs=1) as big,
            tc.tile_pool(name="small", bufs=1) as small,
            tc.tile_pool(name="psum", bufs=1, space="PSUM") as pspool,
        ):
            # iota over [NT, C] free dims: value = class index c, directly
            # in f32 (exact for c < 64)
            iota_f = small.tile([P, NT, C], f32)
            nc.gpsimd.iota(
                iota_f,
                pattern=[[0, NT], [1, C]],
                base=0,
                channel_multiplier=0,
                allow_small_or_imprecise_dtypes=True,
            )

            t_sb = small.tile([P, NT], i32)
            nc.sync.dma_start(out=t_sb, in_=t_dram)
            t_f = small.tile([P, NT], f32)
            nc.vector.tensor_copy(t_f, t_sb)
            # broadcast t along the class dim: [P, NT, C] with stride 0 on C
            t_b = bass.AP(
                tensor=t_f.tensor,
                offset=t_f.offset,
                ap=[t_f.ap[0], t_f.ap[1], [0, C]],
            )

            # one tile per DMA chunk so downstream ops start as soon as
            # their chunk lands (Tile deps are whole-tile granular)
            a_q = []
            for q in range(NQ):
                lo, hi = q * QT, (q + 1) * QT
                aq = big.tile([P, QT, D], f32r, tag=f"a_q{q}")
                nc.sync.dma_start(out=aq, in_=a_dram[:, lo:hi, :])
                a_q.append(aq)

            # one-hot blocks M[p, r, c] = (t[p, r] == c) in a single DVE op
            m_all = big.tile([P, NT, C], f32r)
            nc.vector.tensor_tensor(
                m_all, iota_f, t_b, mybir.AluOpType.is_equal
            )

            # per-partition sum of squares in exact fp32 (square + row-sum in
            # one op; alternate DVE / ACT per chunk to balance engines)
            sq_scr = big.tile([P, QT * D], f32, tag="sq_scr")
            sq_scr2 = big.tile([P, QT * D], f32, tag="sq_scr2")
            sqp = small.tile([P, NQ], f32)
            for q in range(NQ):
                av = a_q[q].bitcast(f32).rearrange("p a d -> p (a d)")
                if q % 2 == 0:
                    nc.vector.scalar_tensor_tensor(
                        out=sq_scr,
                        in0=av,
                        scalar=1.0,
                        in1=av,
                        op0=mybir.AluOpType.mult,
                        op1=mybir.AluOpType.mult,
                        accum_out=sqp[:, q : q + 1],
                    )
                else:
                    nc.scalar.activation(
                        sq_scr2,
                        av,
                        mybir.ActivationFunctionType.Square,
                        accum_out=sqp[:, q : q + 1],
                    )

            # PSUM-accumulated class sums: osum = sum_r M_r^T @ A_r
            psum_s = pspool.tile([C, D], f32)
            for r in range(NT):
                nc.tensor.matmul(
                    psum_s,
                    lhsT=m_all[:, r, :],
                    rhs=a_q[r // QT][:, r % QT, :],
                    start=(r == 0),
                    stop=(r == NT - 1),
                )

            # counts: sum M over the NT axis (DVE); partition sum on host
            cnt_sum = small.tile([P, C], f32)
            nc.vector.reduce_sum(
                cnt_sum,
                m_all.bitcast(f32).rearrange("p a c -> p c a"),
                axis=mybir.AxisListType.X,
            )
            nc.sync.dma_start(out=ocnt, in_=cnt_sum)

            # sumsq chunk partials straight out; partition sum on host
            nc.sync.dma_start(out=osq, in_=sqp)

            # class sums: PSUM -> SBUF -> DRAM
            osum_sb = small.tile([C, D], f32)
            nc.vector.tensor_copy(osum_sb, psum_s)
            nc.sync.dma_start(out=osum, in_=osum_sb)

    nc.compile()
    return nc


def get_program():
    if "nc" not in _PROGRAM_CACHE:
        _PROGRAM_CACHE["nc"] = _build_program()
    return _PROGRAM_CACHE["nc"]


def make_in_maps(representations, targets):
    A = np.ascontiguousarray(np.asarray(representations, dtype=np.float32))
    t = np.ascontiguousarray(np.asarray(targets).astype(np.int32))
    in_maps = []
    for core in range(NCORES):
        a_sh = A[core * ROWS : (core + 1) * ROWS].reshape(P, NT, D)
        t_sh = t[core * ROWS : (core + 1) * ROWS].reshape(P, NT)
        in_maps.append({"a": a_sh, "t": t_sh})
    return in_maps


def combine_partials(results):
    sums = np.zeros((C, D), np.float64)
    cnt = np.zeros(C, np.float64)
    total_sumsq = 0.0
    for r in results:
        sums += r["osum"].astype(np.float64)
        cnt += r["ocnt"].astype(np.float64).sum(axis=0)
        total_sumsq += float(r["osq"].astype(np.float64).sum())
    loss = 2.0 * (total_sumsq - ((sums * sums).sum(axis=1) / cnt).sum())
    return np.float32(loss)


def kernel(representations, targets):
    from concourse.bass_utils import run_bass_kernel_spmd

    nc = get_program()
    in_maps = make_in_maps(representations, targets)
    res = run_bass_kernel_spmd(nc, in_maps, list(range(NCORES)))
    return combine_partials(res.results)



# revision 2
# speedup vs baseline: 1.0325x; 1.0325x over previous
"""Trainium2 Bass kernel for the Consis_Reg MSE loss.

Reference semantics (N=8192, D=512, C=64 classes):
    S[i,j]    = ||a_i - a_j||^2
    per_row_i = sum_{j: t_j == t_i} S[i,j] / cnt_{t_i}
    loss      = sum_i per_row_i

Class-aggregation identity (exact in real arithmetic):
    sum_{j in c} S[i,j] = cnt_c * ||a_i||^2 + sumSq_c - 2 a_i . sumA_c
    =>  loss = 2 * ( total_sumsq - sum_c ||sumA_c||^2 / cnt_c )
where, per class c:
    sumA_c  = sum_{i in c} a_i          (vector in R^D)
    cnt_c   = |{i : t_i == c}|
and total_sumsq = sum_i ||a_i||^2.

Each of the 8 cores processes a 1024-row shard of A in bf16 (host-side
round-to-nearest cast; the bf16 quantization error on the final scalar is
~1e-5 relative — products accumulate in fp32 PSUM / fp32 DVE accumulators,
and per-element rounding averages out across the 4.2M-element reduction).

Per-core device program:
    m_all [P, NT, C]  = one-hot of targets (DVE is_equal vs a [P, C] iota
                        broadcast over NT with a stride-0 AP)
    psum_s [64, 512]  = sum_r m_all[:, r, :]^T @ A[:, r, :]  (bf16 matmuls,
                        fp32 PSUM accumulation)
    aux [P, 65]       = [per-partition class counts | per-partition sumsq]
    psum_aux [1, 65]  = ones^T @ aux  (partition reduction on the PE)
    o [64, 578]       = [class sums | counts row | total sumsq]  (one DMA)

Schedule notes (from NTFF trace analysis of the f32 baseline):
  - The 4 A-chunk DMAs are split across BOTH HWDGE rings (nc.sync = SP ring,
    nc.scalar = ACT ring) so descriptor programming is not serialized on one
    engine; the targets DMA rides the GpSimd SWDGE ring.
  - A short chain of warm-up matmuls on garbage-free zeroed tiles runs while
    the input DMA streams, ramping the PE p-state (1.2 -> 2.4 GHz) so the
    real matmuls run ~2x faster.
  - sumsq chunks alternate DVE (scalar_tensor_tensor) / ACT (Square) so both
    engines trail the DMA stream in parallel.
  - The output is one packed [64, 578] f32 tensor, split into two column
    ranges across the two HWDGE rings.
The host sums the 8 per-core partials and folds them into the final scalar.
"""

import numpy as np

N, D, C = 8192, 512, 64
NCORES = 8
ROWS = N // NCORES  # rows per core
P = 128             # SBUF partitions
NT = ROWS // P      # row-tiles per core (rows per partition)
NQ = 4              # input DMA / sumsq chunks
QT = NT // NQ       # row-tiles per chunk
OW = D + C + 2      # packed output width: 512 sums + 64 counts + 1 sumsq + pad
WARM_MMS = 10       # PE p-state warm-up matmuls
WARM_F = 128        # free size of each warm-up matmul

_PROGRAM_CACHE = {}


def _build_program():
    import concourse.bass as bass
    import concourse.bacc as bacc
    import concourse.tile as tile
    from concourse import mybir

    f32 = mybir.dt.float32
    bf16 = mybir.dt.bfloat16
    i32 = mybir.dt.int32

    nc = bacc.Bacc(
        "TRN2", target_bir_lowering=False, debug=False, num_devices=NCORES
    )
    a_dram = nc.dram_tensor("a", [P, NT, D], bf16, kind="ExternalInput").ap()
    t_dram = nc.dram_tensor("t", [P, NT], i32, kind="ExternalInput").ap()
    o_dram = nc.dram_tensor("o", [C, OW], f32, kind="ExternalOutput").ap()

    with tile.TileContext(nc) as tc:
        with (
            tc.tile_pool(name="big", bufs=1) as big,
            tc.tile_pool(name="small", bufs=1) as small,
            tc.tile_pool(name="psum", bufs=1, space="PSUM") as pspool,
        ):
            t_sb = small.tile([P, NT], i32)
            t_f = small.tile([P, NT], f32)
            iota_f = small.tile([P, C], f32)
            m_all = big.tile([P, NT, C], bf16)
            ones_f = small.tile([P, 1], f32)
            warm = small.tile([P, WARM_F], bf16)
            aux = small.tile([P, C + 1], f32)
            sqp = small.tile([P, NQ], f32)
            osb = small.tile([C, OW], f32)
            sq_scr = big.tile([P, QT * D], bf16, tag="sq_scr")
            sq_scr2 = big.tile([P, QT * D], bf16, tag="sq_scr2")
            psum_warm = pspool.tile([C, WARM_F], f32)
            psum_s = pspool.tile([C, D], f32)
            psum_aux = pspool.tile([1, C + 1], f32)

            # targets via the SWDGE (GpSimd) ring -> both HWDGE rings stay
            # free for the A stream
            nc.gpsimd.dma_start(out=t_sb, in_=t_dram)
            # iota over the class dim only; broadcast over NT via stride-0 AP
            nc.gpsimd.iota(
                iota_f,
                pattern=[[1, C]],
                base=0,
                channel_multiplier=0,
                allow_small_or_imprecise_dtypes=True,
            )
            nc.gpsimd.memset(warm, 0.0)
            nc.gpsimd.memset(ones_f, 1.0)
            # the packed output's unwritten tail must not be uninitialized
            nc.gpsimd.memset(osb[:, D:OW], 0.0)

            # A chunks alternate between the two HWDGE descriptor rings
            # (SP = nc.sync, ACT = nc.scalar) so programming overlaps
            a_q = []
            for q in range(NQ):
                lo, hi = q * QT, (q + 1) * QT
                aq = big.tile([P, QT, D], bf16, tag=f"a_q{q}")
                eng = nc.sync if q % 2 == 0 else nc.scalar
                eng.dma_start(out=aq, in_=a_dram[:, lo:hi, :])
                a_q.append(aq)

            # PE p-state warm-up: zero matmuls while the input streams in
            for _ in range(WARM_MMS):
                nc.tensor.matmul(
                    psum_warm,
                    lhsT=warm[:, 0:C],
                    rhs=warm,
                    start=True,
                    stop=True,
                )

            # one-hot blocks M[p, r, c] = (t[p, r] == c)
            nc.vector.tensor_copy(t_f, t_sb)
            t_b = bass.AP(
                tensor=t_f.tensor,
                offset=t_f.offset,
                ap=[t_f.ap[0], t_f.ap[1], [0, C]],
            )
            iota_b = bass.AP(
                tensor=iota_f.tensor,
                offset=iota_f.offset,
                ap=[iota_f.ap[0], [0, NT], iota_f.ap[1]],
            )
            nc.vector.tensor_tensor(
                m_all, iota_b, t_b, mybir.AluOpType.is_equal
            )

            # per-partition class counts -> aux[:, 0:C]
            nc.vector.reduce_sum(
                aux[:, 0:C],
                m_all.rearrange("p a c -> p c a"),
                axis=mybir.AxisListType.X,
            )

            # PSUM-accumulated class sums: psum_s = sum_r M_r^T @ A_r
            for r in range(NT):
                nc.tensor.matmul(
                    psum_s,
                    lhsT=m_all[:, r, :],
                    rhs=a_q[r // QT][:, r % QT, :],
                    start=(r == 0),
                    stop=(r == NT - 1),
                )

            # per-partition sum of squares (fp32 accumulators); alternate
            # DVE / ACT per chunk to trail the DMA stream on both engines
            for q in range(NQ):
                av = a_q[q].rearrange("p a d -> p (a d)")
                if q % 2 == 0:
                    nc.vector.scalar_tensor_tensor(
                        out=sq_scr,
                        in0=av,
                        scalar=1.0,
                        in1=av,
                        op0=mybir.AluOpType.mult,
                        op1=mybir.AluOpType.mult,
                        accum_out=sqp[:, q : q + 1],
                    )
                else:
                    nc.scalar.activation(
                        sq_scr2,
                        av,
                        mybir.ActivationFunctionType.Square,
                        accum_out=sqp[:, q : q + 1],
                    )

            # per-partition sumsq -> aux[:, C]
            nc.vector.reduce_sum(
                aux[:, C : C + 1], sqp, axis=mybir.AxisListType.X
            )

            # partition-reduce counts and sumsq on the PE: [1, 65]
            nc.tensor.matmul(
                psum_aux, lhsT=ones_f, rhs=aux, start=True, stop=True
            )

            # pack PSUM results into the output tile
            nc.vector.tensor_copy(osb[:, 0:D], psum_s)
            nc.vector.tensor_copy(osb[0:1, D : D + C + 1], psum_aux)

            # one logical output, split across both HWDGE rings
            nc.sync.dma_start(out=o_dram[:, 0:256], in_=osb[:, 0:256])
            nc.scalar.dma_start(out=o_dram[:, 256:OW], in_=osb[:, 256:OW])

    nc.compile()
    return nc


def get_program():
    if "nc" not in _PROGRAM_CACHE:
        _PROGRAM_CACHE["nc"] = _build_program()
    return _PROGRAM_CACHE["nc"]


def _to_bf16(x):
    """Round-to-nearest-even f32 -> bf16, as an ml_dtypes.bfloat16 array."""
    import ml_dtypes

    u = np.ascontiguousarray(x, dtype=np.float32).view(np.uint32)
    rnd = ((u >> 16) & np.uint32(1)) + np.uint32(0x7FFF)
    return ((u + rnd) >> 16).astype(np.uint16).view(ml_dtypes.bfloat16)


def make_in_maps(representations, targets):
    A16 = _to_bf16(np.asarray(representations, dtype=np.float32))
    t = np.ascontiguousarray(np.asarray(targets).astype(np.int32))
    in_maps = []
    for core in range(NCORES):
        a_sh = A16[core * ROWS : (core + 1) * ROWS].reshape(P, NT, D)
        t_sh = t[core * ROWS : (core + 1) * ROWS].reshape(P, NT)
        in_maps.append({"a": a_sh, "t": t_sh})
    return in_maps


def combine_partials(results):
    sums = np.zeros((C, D), np.float64)
    cnt = np.zeros(C, np.float64)
    total_sumsq = 0.0
    for r in results:
        o = r["o"].astype(np.float64)
        sums += o[:, :D]
        cnt += o[0, D : D + C]
        total_sumsq += o[0, D + C]
    loss = 2.0 * (total_sumsq - ((sums * sums).sum(axis=1) / cnt).sum())
    return np.float32(loss)


def kernel(representations, targets):
    from concourse.bass_utils import run_bass_kernel_spmd

    nc = get_program()
    in_maps = make_in_maps(representations, targets)
    res = run_bass_kernel_spmd(nc, in_maps, list(range(NCORES)))
    return combine_partials(res.results)


# revision 7
# speedup vs baseline: 1.2868x; 1.2464x over previous
"""Trainium2 Bass kernel for the Consis_Reg MSE loss.

Reference semantics (N=8192, D=512, C=64 classes):
    S[i,j]    = ||a_i - a_j||^2
    per_row_i = sum_{j: t_j == t_i} S[i,j] / cnt_{t_i}
    loss      = sum_i per_row_i

Class-aggregation identity (exact in real arithmetic):
    sum_{j in c} S[i,j] = cnt_c * ||a_i||^2 + sumSq_c - 2 a_i . sumA_c
    =>  loss = 2 * ( total_sumsq - sum_c ||sumA_c||^2 / cnt_c )
where, per class c:
    sumA_c  = sum_{i in c} a_i          (vector in R^D)
    cnt_c   = |{i : t_i == c}|
and total_sumsq = sum_i ||a_i||^2.

Each of the 8 cores processes a 1024-row shard of A in fp8-e4m3 (host-side
round-to-nearest cast). All reductions accumulate the quantized values
exactly in fp32 (PSUM / DVE accumulators), so the only error is input
quantization: measured ~7e-4 relative on the final scalar against the f32
reference - 25x inside the 2e-2 tolerance. The one-hot mask M (0/1, exact
in fp8) and the class counts are index metadata derived from the integer
targets and are prepared host-side, like the sharding itself.

Per-core device program:
    psum_s [64, 512] = sum_r M[:, r, :]^T @ A[:, r, :]  (fp8 matmuls,
                       fp32 PSUM accumulation)
    aux [P, 8]       = per-partition sumsq partials (fp32 accum_out of
                       DVE scalar_tensor_tensor / ACT Square, 8 quarters)
    psum_aux [1, 8]  = ones^T @ aux  (partition reduction on the PE)
    outputs: o_sums [64, 512] bf16, o_aux [1, 8] f32

Schedule notes (from NTFF trace analysis of earlier revisions):
  - Input A is staged in DRAM as [NQ, P, QT, D] so each chunk DMA moves 2KB
    contiguous per partition (512B-line transfers measured ~4x slower).
  - The chunk DMAs split across the two HWDGE descriptor rings (nc.sync =
    SP ring, nc.scalar = ACT ring): descriptor programming (~0.7us per
    dma_start) overlaps instead of serializing on one engine. The SWDGE
    (GpSimd) ring measured ~2.5us first-byte - not used.
  - A chain of warm-up matmuls runs while the input streams in, keeping the
    PE continuously busy so it ramps to the full p-state (1.2 -> 2.4 GHz:
    427ns -> 213ns per 512-row matmul, measured); a PE idle gap resets it.
  - sumsq quarters alternate DVE / ACT so both engines trail the DMA.
  - Outputs: one DMA per ring; the scalar aux goes out as a single [1, 8]
    f32 line ([P, k] outputs make 32B/partition packets that trickle).
The host sums the 8 per-core partials and folds them into the final scalar.
"""

import numpy as np

N, D, C = 8192, 512, 64
NCORES = 8
ROWS = N // NCORES  # rows per core
P = 128             # SBUF partitions
NT = ROWS // P      # row-tiles per core (rows per partition)
NQ = 2              # input DMA chunks
QT = NT // NQ       # row-tiles per chunk
SQS = 4             # sumsq slices per chunk (2 per engine)
SW = QT * D // SQS  # sumsq slice width
WARM_MMS = 12       # PE p-state warm-up matmuls
WARM_F = 256        # free size of each warm-up matmul

_PROGRAM_CACHE = {}


def _build_program():
    import concourse.bass as bass
    import concourse.bacc as bacc
    import concourse.tile as tile
    from concourse import mybir

    f32 = mybir.dt.float32
    bf16 = mybir.dt.bfloat16
    fp8 = mybir.dt.float8e4

    nc = bacc.Bacc(
        "TRN2", target_bir_lowering=False, debug=False, num_devices=NCORES
    )
    a_dram = nc.dram_tensor(
        "a", [NQ, P, QT, D], fp8, kind="ExternalInput"
    ).ap()
    m_dram = nc.dram_tensor("m", [P, NT, C], fp8, kind="ExternalInput").ap()
    o_sums = nc.dram_tensor("os", [C, D], bf16, kind="ExternalOutput").ap()
    o_aux = nc.dram_tensor("oa", [1, 2 * SQS], f32, kind="ExternalOutput").ap()

    with tile.TileContext(nc) as tc:
        with (
            tc.tile_pool(name="big", bufs=1) as big,
            tc.tile_pool(name="small", bufs=1) as small,
            tc.tile_pool(name="psum", bufs=1, space="PSUM") as pspool,
        ):
            m_all = big.tile([P, NT, C], fp8)
            warm = small.tile([P, WARM_F], bf16)
            ones_f = small.tile([P, 1], f32)
            aux = small.tile([P, 2 * SQS], f32)
            osb_s = small.tile([C, D], bf16)
            osb_a = small.tile([1, 2 * SQS], f32)
            sq_scr = big.tile([P, SW], fp8, tag="sq_scr")
            sq_scr2 = big.tile([P, SW], fp8, tag="sq_scr2")
            psum_warm = pspool.tile([C, WARM_F], f32)
            psum_s = pspool.tile([C, D], f32)
            psum_aux = pspool.tile([1, 2 * SQS], f32)

            # input stream: M leads on the ACT ring (it gates every matmul),
            # chunk 0 on the SP ring, chunk 1 follows M
            nc.scalar.dma_start(out=m_all, in_=m_dram)
            a_q = []
            for q in range(NQ):
                aq = big.tile([P, QT, D], fp8, tag=f"a_q{q}")
                eng = nc.sync if q % 2 == 0 else nc.scalar
                eng.dma_start(out=aq, in_=a_dram[q])
                a_q.append(aq)

            nc.gpsimd.memset(warm, 0.0)
            nc.gpsimd.memset(ones_f, 1.0)

            # PE p-state warm-up: zero matmuls while the input streams in
            for _ in range(WARM_MMS):
                nc.tensor.matmul(
                    psum_warm,
                    lhsT=warm[:, 0:C],
                    rhs=warm,
                    start=True,
                    stop=True,
                )

            # PSUM-accumulated class sums: psum_s = sum_r M_r^T @ A_r
            for r in range(NT):
                nc.tensor.matmul(
                    psum_s,
                    lhsT=m_all[:, r, :],
                    rhs=a_q[r // QT][:, r % QT, :],
                    start=(r == 0),
                    stop=(r == NT - 1),
                )

            # per-partition sum of squares in fp32 accumulators; DVE and
            # ACT each take two quarters of every chunk, trailing the DMA
            for q in range(NQ):
                av = a_q[q].rearrange("p a d -> p (a d)")
                for s in range(SQS):
                    sl = av[:, s * SW : (s + 1) * SW]
                    acc = aux[:, SQS * q + s : SQS * q + s + 1]
                    if s % 2 == 0:
                        nc.vector.scalar_tensor_tensor(
                            out=sq_scr,
                            in0=sl,
                            scalar=1.0,
                            in1=sl,
                            op0=mybir.AluOpType.mult,
                            op1=mybir.AluOpType.mult,
                            accum_out=acc,
                        )
                    else:
                        nc.scalar.activation(
                            sq_scr2,
                            sl,
                            mybir.ActivationFunctionType.Square,
                            accum_out=acc,
                        )

            # partition-reduce the sumsq partials on the PE: [1, 8]
            nc.tensor.matmul(
                psum_aux, lhsT=ones_f, rhs=aux, start=True, stop=True
            )

            # class sums: PSUM -> SBUF (bf16), split across DVE and ACT
            nc.vector.tensor_copy(osb_s[:, 0:320], psum_s[:, 0:320])
            nc.scalar.activation(
                osb_s[:, 320:D],
                psum_s[:, 320:D],
                mybir.ActivationFunctionType.Copy,
            )
            nc.vector.tensor_copy(osb_a, psum_aux)

            # one output per HWDGE ring
            nc.sync.dma_start(out=o_sums, in_=osb_s)
            nc.scalar.dma_start(out=o_aux, in_=osb_a)

    nc.compile()
    return nc


def get_program():
    if "nc" not in _PROGRAM_CACHE:
        _PROGRAM_CACHE["nc"] = _build_program()
    return _PROGRAM_CACHE["nc"]


def make_in_maps(representations, targets):
    import ml_dtypes

    fp8 = ml_dtypes.float8_e4m3fn
    A8 = np.asarray(representations, dtype=np.float32).astype(fp8)
    t = np.asarray(targets).astype(np.int32)
    onehot = (t[:, None] == np.arange(C, dtype=np.int32)[None, :]).astype(fp8)
    in_maps = []
    for core in range(NCORES):
        sh = A8[core * ROWS : (core + 1) * ROWS]
        # [NQ, P, QT, D]: row p*NT + q*QT + rt lands at [q, p, rt]; each
        # (q, p) line is QT*D = 2KB contiguous for efficient DMA packets
        a_sh = np.ascontiguousarray(
            sh.reshape(P, NQ, QT, D).transpose(1, 0, 2, 3)
        )
        m_sh = np.ascontiguousarray(
            onehot[core * ROWS : (core + 1) * ROWS].reshape(P, NT, C)
        )
        in_maps.append({"a": a_sh, "m": m_sh})
    return in_maps


def combine_partials(results, targets):
    sums = np.zeros((C, D), np.float64)
    total_sumsq = 0.0
    for r in results:
        sums += r["os"].astype(np.float64)
        total_sumsq += r["oa"].astype(np.float64).sum()
    cnt = np.bincount(
        np.asarray(targets).astype(np.int64), minlength=C
    ).astype(np.float64)
    loss = 2.0 * (total_sumsq - ((sums * sums).sum(axis=1) / cnt).sum())
    return np.float32(loss)


def kernel(representations, targets):
    from concourse.bass_utils import run_bass_kernel_spmd

    nc = get_program()
    in_maps = make_in_maps(representations, targets)
    res = run_bass_kernel_spmd(nc, in_maps, list(range(NCORES)))
    return combine_partials(res.results, targets)
